# revision 47
# baseline (speedup 1.0000x reference)
"""Trainium2 Bass kernel for a dense transformer self-attention block.

Problem: out = (softmax(QK^T/sqrt(dk) + mask) V) Wo + bo  with fused QKV proj.
  x [2, 2048, 1024], 16 heads, dk=64. Returns (out, attn).

Sharding (8 cores): core c handles batch b=c//4 and head-group g=c%4
(heads 4g..4g+3).  Pure data/tensor parallelism: no collectives; the
host sums the 4 output-projection partials per batch and reassembles
the attention probabilities (pure layout: transpose + dtype cast).

Device dataflow per core (all matmul operands bf16, accumulation f32):
  phase 1: qkT = [Wq'|Wk]^T x^T   (Wq' pre-scaled by 1/sqrt(dk) on host)
           V   = x Wv             (natural [k, dv] layout, + ones column)
  main, per (q-chunk, head-pair):
    scoresT[k,q] = K Q^T          (two heads packed in the PE array via
                                   row groups: dk=64 contraction each)
    E = exp(scoresT)              (ScalarE, straight from PSUM; max-
                                   subtraction skipped: |scores| <~ 30)
    blendT[dv+1, q] = [V|1]^T E   (ones column makes row 64 the softmax
                                   denominator - no partition reduce)
    r = 1/denom broadcast to 128 partitions (GpSimd)
    attn = E * r  -> HBM as [h, k, q] bf16 (host transposes to [q, k])
    blend = blendT * r
  out_partial[s, :] = blend^T Wo  -> HBM f32
"""

import os
import numpy as np
import ml_dtypes
from contextlib import ExitStack

B, S, D, H, DK = 2, 2048, 1024, 16, 64
NCORES = 8
HPC = 4  # heads per core
NEG_BIG = -float(2**63)
BF16 = ml_dtypes.bfloat16

PROFILE = False  # set True (e.g. from test.py) to neuron-profile the run
TRACE_DIR = None
LAST_EXEC_NS = None
LAST_RESULTS = None

_GRAPH_CACHE = {}


def _build_graph(use_mask, use_qk_bias, use_v_bias, debug_taps=False):
    import concourse.bass as bass
    import concourse.mybir as mybir
    import concourse.tile as tile
    from concourse import bacc

    DT = mybir.dt
    BF = DT.bfloat16
    F32 = DT.float32

    nc = bacc.Bacc("TRN2", target_bir_lowering=False, debug=False,
                   num_devices=NCORES)

    xt_d = nc.dram_tensor("xt", [D, S], BF, kind="ExternalInput")
    wqk_d = nc.dram_tensor("wqk", [D, 512], BF, kind="ExternalInput")
    wv_d = nc.dram_tensor("wv", [D, 256], BF, kind="ExternalInput")
    wo_d = nc.dram_tensor("wo", [256, D], BF, kind="ExternalInput")
    if use_mask:
        amb_d = nc.dram_tensor("amb", [128, 16], F32, kind="ExternalInput")
    if use_qk_bias:
        qkb_d = nc.dram_tensor("qkb", [128, 4], F32, kind="ExternalInput")
    if use_v_bias:
        vb_d = nc.dram_tensor("vb", [64, 4], F32, kind="ExternalInput")
    attn_d = nc.dram_tensor("attn_t", [HPC, S, S], BF, kind="ExternalOutput")
    outp_d = nc.dram_tensor("out_p", [S, D], F32, kind="ExternalOutput")
    if debug_taps:
        dbg_qkt = nc.dram_tensor("dbg_qkt", [4, 128, S], BF, kind="ExternalOutput")
        dbg_v = nc.dram_tensor("dbg_v", [128, 16, 264], BF, kind="ExternalOutput")
        dbg_exp = nc.dram_tensor("dbg_exp", [128, 16, 1024], BF,
                                 kind="ExternalOutput")
        dbg_den = nc.dram_tensor("dbg_den", [4, 512], F32, kind="ExternalOutput")
        dbg_rbc = nc.dram_tensor("dbg_rbc", [4, 128, 512], BF,
                                 kind="ExternalOutput")
        dbg_bl = nc.dram_tensor("dbg_bl", [4, 64, 512], BF, kind="ExternalOutput")

    EXP = mybir.ActivationFunctionType.Exp
    MUL = mybir.AluOpType.mult
    ADD = mybir.AluOpType.add

    with tile.TileContext(nc) as tc, ExitStack() as ctx:
        consts = ctx.enter_context(tc.tile_pool(name="consts", bufs=1))
        work = ctx.enter_context(tc.tile_pool(name="work", bufs=3))
        small = ctx.enter_context(tc.tile_pool(name="small", bufs=3))
        outs = ctx.enter_context(tc.tile_pool(name="outs", bufs=3))
        blpool = ctx.enter_context(tc.tile_pool(name="bl", bufs=2))
        xtp = ctx.enter_context(tc.tile_pool(name="xtp", bufs=24))
        # PSUM budget (8 banks): scores 2x2 + blend(psb) 3x1 + outproj 1 = 8
        # (phase-1 and broadcast matmuls borrow the scores slots)
        ps_s = ctx.enter_context(tc.tile_pool(name="ps_s", bufs=2, space="PSUM"))
        ps_b = ctx.enter_context(tc.tile_pool(name="ps_b", bufs=3, space="PSUM"))
        ps_x = ctx.enter_context(tc.tile_pool(name="ps_x", bufs=1, space="PSUM"))

        # ---- constant loads (issue order matters: first matmul group needs
        # wqk + wv + the sc=0 x^T chunks only) ---------------------------------
        wqk_sb = consts.tile([128, 8, 512], BF, tag="wqk")
        nc.sync.dma_start(wqk_sb[:], wqk_d.ap().rearrange("(o p) m -> p o m", p=128))
        wv_sb = consts.tile([128, 8, 256], BF, tag="wv")
        nc.sync.dma_start(wv_sb[:], wv_d.ap().rearrange("(o p) m -> p o m", p=128))
        # x^T in [dc][sc] chunks of [128, 512] so phase-1 matmuls can start
        # as soon as the first s-chunk lands
        xt_sb = {}
        for sc in range(4):
            for dc in range(8):
                t = xtp.tile([128, 512], BF, tag="xt", name=f"xt{dc}_{sc}")
                nc.sync.dma_start(t[:], xt_d.ap()[dc * 128:(dc + 1) * 128,
                                                  sc * 512:(sc + 1) * 512])
                xt_sb[(dc, sc)] = t
        wo_sb = consts.tile([128, 2, 1024], BF, tag="wo")
        nc.sync.dma_start(wo_sb[:], wo_d.ap().rearrange("(o p) m -> p o m", p=128))
        if use_mask:
            amb_sb = consts.tile([128, 16], F32, tag="amb")
            nc.sync.dma_start(amb_sb[:], amb_d.ap())
        if use_qk_bias:
            qkb_sb = consts.tile([128, 4], F32, tag="qkb")
            nc.sync.dma_start(qkb_sb[:], qkb_d.ap())
        if use_v_bias:
            vb_sb = consts.tile([64, 4], F32, tag="vb")
            nc.sync.dma_start(vb_sb[:], vb_d.ap())

        qT_sb = consts.tile([128, 2, S], BF, tag="qT")
        kT_sb = consts.tile([128, 2, S], BF, tag="kT")
        # v_aug layout: per head a 66-col strip: [V(64) | ones | pad]
        v_sb = consts.tile([128, 16, 264], BF, tag="v")
        nc.vector.memset(v_sb[:], 1.0)
        ones_sb = consts.tile([128, 128], BF, tag="ones")
        nc.vector.memset(ones_sb[:], 1.0)

        # ---- emission helpers ----------------------------------------------
        def qkT_group(mt, sc):
            """One [128, 512] chunk of qkT = [Wq'|Wk]^T x^T."""
            target = qT_sb if mt < 2 else kT_sb
            prr = mt % 2
            ps = ps_s.tile([128, 512], F32, tag="scores", name=f"p1_{mt}_{sc}")
            for dc in range(8):
                nc.tensor.matmul(
                    ps[:],
                    lhsT=wqk_sb[:, dc, mt * 128:(mt + 1) * 128],
                    rhs=xt_sb[(dc, sc)][:, :],
                    start=(dc == 0), stop=(dc == 7))
            dst = target[:, prr, sc * 512:(sc + 1) * 512]
            if use_qk_bias:
                nc.vector.tensor_scalar(
                    out=dst, in0=ps[:], scalar1=qkb_sb[:, mt:mt + 1],
                    scalar2=None, op0=ADD)
            else:
                nc.vector.tensor_copy(out=dst, in_=ps[:])

        def v_group(st):
            """One [128 k, 256] s-tile of V = x Wv into the v_aug layout."""
            ps = ps_s.tile([128, 256], F32, tag="scores", name=f"p1v_{st}")
            for dc in range(8):
                nc.tensor.matmul(
                    ps[:],
                    lhsT=xt_sb[(dc, st // 4)][:, (st % 4) * 128:(st % 4 + 1) * 128],
                    rhs=wv_sb[:, dc, :],
                    start=(dc == 0), stop=(dc == 7))
            dst = v_sb[:, st, :].rearrange("p (h c) -> p h c", h=4)[:, :, 0:64]
            srcv = ps[:].rearrange("p (h c) -> p h c", h=4)
            nc.vector.tensor_copy(out=dst, in_=srcv)

        def pair_scores_exp(qc, pr, attn_un, kc):
            """scoresT pair matmul (row-group packed) + exp for one k-chunk."""
            ps = ps_s.tile([128, 1024], F32, tag="scores",
                           name=f"sc_{qc}_{pr}_{kc}")
            for m in range(2):
                nc.tensor.matmul(
                    ps[:, m * 512:(m + 1) * 512],
                    lhsT=kT_sb[m * 64:(m + 1) * 64, pr,
                               kc * 128:(kc + 1) * 128],
                    rhs=qT_sb[m * 64:(m + 1) * 64, pr,
                              qc * 512:(qc + 1) * 512],
                    start=True, stop=True)
            bias = amb_sb[:, kc:kc + 1] if use_mask else 0.0
            nc.scalar.activation(out=attn_un[:, kc, :], in_=ps[:],
                                 func=EXP, bias=bias)

        def pair_av(pr, attn_un, psb, kc):
            """attn@V accumulation (V augmented with a ones column) for one
            k-chunk of a pair."""
            for m in range(2):
                h = 2 * pr + m
                nc.tensor.matmul(
                    psb[m][0:65, :],
                    lhsT=v_sb[:, kc, h * 66:h * 66 + 65],
                    rhs=attn_un[:, kc, m * 512:(m + 1) * 512],
                    start=(kc == 0), stop=(kc == 15))

        def pair_epilogue(qc, pr, attn_un, psb):
            """Denominator -> reciprocal broadcast; normalize + emit attn and
            blend.  Returns the pair's blend tile [128 hdv, 512 q]."""
            bp = blpool.tile([128, 512], BF, tag="bp", name=f"bp_{qc}_{pr}")
            # tiny lane-64 casts first so the PE broadcast matmuls are never
            # stuck behind a multi-us TENSOR_TENSOR on VectorE
            rbs, rbcs = [], []
            for m in range(2):
                rb = small.tile([128, 512], BF, tag="rb")
                nc.vector.tensor_copy(out=rb[64:65, :], in_=psb[m][64:65, :])
                rbs.append(rb)
            rbp = ps_s.tile([128, 1024], F32, tag="scores", name="rbp")
            for m in range(2):
                nc.tensor.matmul(rbp[:, m * 512:(m + 1) * 512],
                                 lhsT=ones_sb[64:65, 0:128],
                                 rhs=rbs[m][64:65, :], start=True, stop=True)
            for m in range(2):
                rc = small.tile([128, 512], F32, tag="rc")
                nc.vector.reciprocal_approx_fast(
                    out=rc[:, :], in_=rbp[:, m * 512:(m + 1) * 512])
                rbc = small.tile([128, 512], BF, tag="rbc")
                nc.vector.tensor_copy(out=rbc[:, :], in_=rc[:, :])
                rbcs.append(rbc)
                if debug_taps and qc == 0:
                    nc.sync.dma_start(out=dbg_den.ap()[2 * pr + m:
                                                       2 * pr + m + 1, :],
                                      in_=rc[0:1, :])
                    nc.sync.dma_start(out=dbg_rbc.ap()[2 * pr + m, :, :],
                                      in_=rbc[:, :])
            for m in range(2):
                h = 2 * pr + m
                rbc = rbcs[m]
                # normalize attn tiles in place, then write out (gpsimd queue
                # so bulk attn traffic never blocks small control DMAs)
                for half in range(2):
                    sl = attn_un[:, half * 8:(half + 1) * 8,
                                 m * 512:(m + 1) * 512]
                    nc.vector.tensor_tensor(
                        out=sl, in0=sl,
                        in1=rbc[:, None, :].to_broadcast((128, 8, 512)),
                        op=MUL)
                    nc.sync.dma_start(
                        out=attn_d.ap()[h].rearrange("(t p) q -> p t q", p=128)
                        [:, half * 8:(half + 1) * 8,
                         qc * 512:(qc + 1) * 512],
                        in_=sl)
                # normalize blended values (f32 from PSUM, -> bf16)
                if m == 0:
                    nc.vector.tensor_tensor(out=bp[0:64, :], in0=psb[m][0:64, :],
                                            in1=rbc[0:64, :], op=MUL)
                    if use_v_bias:
                        nc.vector.tensor_scalar(
                            out=bp[0:64, :], in0=bp[0:64, :],
                            scalar1=vb_sb[:, h:h + 1], scalar2=None, op0=ADD)
                else:
                    bt = small.tile([64, 512], BF, tag="btmp")
                    nc.vector.tensor_tensor(out=bt[:], in0=psb[m][0:64, :],
                                            in1=rbc[0:64, :], op=MUL)
                    if use_v_bias:
                        nc.vector.tensor_scalar(
                            out=bt[:], in0=bt[:], scalar1=vb_sb[:, h:h + 1],
                            scalar2=None, op0=ADD)
                    # lane shift to partitions 64-127 via DMA
                    nc.sync.dma_start(out=bp[64:128, :], in_=bt[:])
                if debug_taps and qc == 0:
                    nc.sync.dma_start(out=dbg_bl.ap()[h, :, :],
                                      in_=bp[m * 64:(m + 1) * 64, :])
            return bp

        def outproj_st(qc, bps, st):
            s0 = qc * 512 + st * 128
            out_sb = outs.tile([128, 1024], F32, tag="out")
            for nk in range(2):
                pso = ps_x.tile([128, 512], F32, tag="aux", name="pso")
                for prr in range(2):
                    nc.tensor.matmul(
                        pso[:],
                        lhsT=bps[prr][:, st * 128:(st + 1) * 128],
                        rhs=wo_sb[:, prr, nk * 512:(nk + 1) * 512],
                        start=(prr == 0), stop=(prr == 1))
                nc.vector.tensor_copy(out=out_sb[:, nk * 512:(nk + 1) * 512],
                                      in_=pso[:])
            nc.sync.dma_start(out=outp_d.ap()[s0:s0 + 128, :], in_=out_sb[:])

        def outproj(qc, bps):
            for st in range(4):
                s0 = qc * 512 + st * 128
                out_sb = outs.tile([128, 1024], F32, tag="out")
                for nk in range(2):
                    pso = ps_x.tile([128, 512], F32, tag="aux", name="pso")
                    for prr in range(2):
                        nc.tensor.matmul(
                            pso[:],
                            lhsT=bps[prr][:, st * 128:(st + 1) * 128],
                            rhs=wo_sb[:, prr, nk * 512:(nk + 1) * 512],
                            start=(prr == 0), stop=(prr == 1))
                    nc.vector.tensor_copy(out=out_sb[:, nk * 512:(nk + 1) * 512],
                                          in_=pso[:])
                nc.sync.dma_start(out=outp_d.ap()[s0:s0 + 128, :], in_=out_sb[:])

        # ---- software pipeline over the 8 (qc, pr) pairs --------------------
        # scores+exp of pair i run per-k-chunk-interleaved with attn@V of pair
        # i-1, so ScalarE's exp stream never waits for TensorE streaming or
        # the pair epilogues.  Pair 0 is additionally interleaved into phase 1
        # (its attn@V fused since the blend accumulators are free), and pair
        # 1's scores+exp fill ScalarE during the trailing phase-1 chunks.
        pairs = [(qc, pr) for qc in range(4) for pr in range(2)]
        au = {}
        psbs = {}
        bps = {}

        au[0] = work.tile([128, 16, 1024], BF, tag="attn_un", name="au0")
        psb_a0 = ps_b.tile([128, 512], F32, tag="blend_ps")
        psb_b0 = ps_b.tile([128, 512], F32, tag="blend_ps")
        psbs[0] = [psb_a0, psb_b0]
        au[1] = work.tile([128, 16, 1024], BF, tag="attn_un", name="au1")
        for sc in range(4):
            qkT_group(2, sc)
            qkT_group(3, sc)
            for st in range(sc * 4, sc * 4 + 4):
                v_group(st)
            if sc == 0:
                qkT_group(0, 0)
                qkT_group(1, 0)
            for kc in range(sc * 4, sc * 4 + 4):
                pair_scores_exp(0, 0, au[0], kc)
                if sc > 0:
                    # pair0's attn@V lags one s-chunk
                    pair_av(0, au[0], psbs[0], kc - 4)
            for kc in range(sc * 4, sc * 4 + 4):
                pair_scores_exp(0, 1, au[1], kc)
        for kc in range(12, 16):
            pair_av(0, au[0], psbs[0], kc)
        qkT_group(0, 1)
        qkT_group(1, 1)
        if debug_taps:
            for mt in range(4):
                tgt = qT_sb if mt < 2 else kT_sb
                nc.sync.dma_start(out=dbg_qkt.ap()[mt, :, :],
                                  in_=tgt[:, mt % 2, :])
            nc.sync.dma_start(out=dbg_v.ap()[:, :, :], in_=v_sb[:])

        bps[0] = pair_epilogue(0, 0, au[0], psbs[0])

        # block 2: scores/exp of pair 2 with the full attn@V of pair 1
        psb_a1 = ps_b.tile([128, 512], F32, tag="blend_ps")
        psb_b1 = ps_b.tile([128, 512], F32, tag="blend_ps")
        psbs[1] = [psb_a1, psb_b1]
        au[2] = work.tile([128, 16, 1024], BF, tag="attn_un", name="au2")
        for kc in range(16):
            pair_scores_exp(1, 0, au[2], kc)
            pair_av(pairs[1][1], au[1], psbs[1], kc)
        bps[1] = pair_epilogue(0, 1, au[1], psbs[1])
        outproj(0, [bps[0], bps[1]])

        # blocks 3..8 with half-block lag: in block i, pair i-1 finishes its
        # attn@V during the first half (epilogue + attn DMA fire mid-block),
        # pair i starts its own attn@V in the second half.
        def alloc_psb(idx):
            psb_a = ps_b.tile([128, 512], F32, tag="blend_ps",
                              name=f"psba{idx}")
            psb_b = ps_b.tile([128, 512], F32, tag="blend_ps",
                              name=f"psbb{idx}")
            psbs[idx] = [psb_a, psb_b]

        for i in range(3, 9):
            live = i < 8
            if live:
                qc, pr = pairs[i]
                au[i] = work.tile([128, 16, 1024], BF, tag="attn_un",
                                  name=f"au{i}")
            for j in range(16):
                if live:
                    pair_scores_exp(qc, pr, au[i], j)
                if j < 8:
                    # finish pair i-1's attn@V
                    if i == 3:
                        if j == 0:
                            alloc_psb(2)
                        pair_av(pairs[2][1], au[2], psbs[2], 2 * j)
                        pair_av(pairs[2][1], au[2], psbs[2], 2 * j + 1)
                    else:
                        pair_av(pairs[i - 1][1], au[i - 1], psbs[i - 1], j + 8)
                else:
                    if j == 8:
                        pqc, ppr = pairs[i - 1]
                        bps[i - 1] = pair_epilogue(pqc, ppr, au[i - 1],
                                                   psbs[i - 1])
                        if ppr == 1 and not live:
                            outproj(pqc, [bps[i - 2], bps[i - 1]])
                    if (live and pairs[i - 1][1] == 1 and j >= 8
                            and j % 2 == 0):
                        # spread the out-proj matmuls so they never delay the
                        # next scores chunk by a full burst
                        outproj_st(pairs[i - 1][0],
                                   [bps[i - 2], bps[i - 1]], (j - 8) // 2)
                    if live:
                        if j == 8:
                            alloc_psb(i)
                        pair_av(pr, au[i], psbs[i], j - 8)
                if live and j == 12 and i in (3, 5):
                    # lazily produce qT for the next q-chunk (PE slack)
                    qkT_group(0, (i + 1) // 2)
                    qkT_group(1, (i + 1) // 2)

    nc.compile()
    return nc


def _prep_inputs(x, attention_mask, wk_w, wk_b, wo_w):
    """Shard + lay out the full inputs for the 8 cores (host-side numpy)."""
    x = np.asarray(x, np.float32)
    wk_w = np.asarray(wk_w, np.float32)
    wk_b = np.asarray(wk_b, np.float32)
    wo_w = np.asarray(wo_w, np.float32)
    am = np.asarray(attention_mask, np.float32)

    amb = 1.0 - am.reshape(B, S)
    amb = np.where(amb == 1.0, np.float32(NEG_BIG), amb).astype(np.float32)
    use_mask = bool(np.any(amb != 0.0))
    use_qk_bias = bool(np.any(wk_b[:2 * D] != 0.0))
    use_v_bias = bool(np.any(wk_b[2 * D:] != 0.0))

    scale = np.float32(1.0 / np.sqrt(DK))
    xt = [np.ascontiguousarray(x[b].T).astype(BF16) for b in range(B)]

    in_maps = []
    for c in range(NCORES):
        b, g = divmod(c, 4)
        cols = slice(g * 256, (g + 1) * 256)
        wq = wk_w[:, cols] * scale
        wk = wk_w[:, 1024:2048][:, cols]
        wv = wk_w[:, 2048:3072][:, cols]
        m = {
            "xt": xt[b],
            "wqk": np.ascontiguousarray(
                np.concatenate([wq, wk], axis=1)).astype(BF16),
            "wv": np.ascontiguousarray(wv).astype(BF16),
            "wo": np.ascontiguousarray(wo_w[g * 256:(g + 1) * 256, :]).astype(BF16),
        }
        if use_mask:
            m["amb"] = np.ascontiguousarray(
                amb[b].reshape(16, 128).T).astype(np.float32)
        if use_qk_bias:
            qb = wk_b[cols] * scale
            kb = wk_b[1024:2048][cols]
            m["qkb"] = np.ascontiguousarray(np.stack(
                [qb[:128], qb[128:], kb[:128], kb[128:]], axis=1)).astype(np.float32)
        if use_v_bias:
            vb = wk_b[2048:3072][cols]
            m["vb"] = np.ascontiguousarray(
                vb.reshape(4, 64).T).astype(np.float32)
        in_maps.append(m)
    return in_maps, (use_mask, use_qk_bias, use_v_bias)


def kernel(x, attention_mask, wk_w, wk_b, wo_w, wo_b):
    global LAST_EXEC_NS, LAST_RESULTS
    from concourse.bass_utils import run_bass_kernel_spmd

    in_maps, variant = _prep_inputs(x, attention_mask, wk_w, wk_b, wo_w)
    if variant not in _GRAPH_CACHE:
        _GRAPH_CACHE[variant] = _build_graph(*variant)
    nc = _GRAPH_CACHE[variant]

    kw = {}
    if PROFILE and TRACE_DIR:
        kw["tmpdir"] = TRACE_DIR
    res = run_bass_kernel_spmd(nc, in_maps, core_ids=list(range(NCORES)),
                               trace=PROFILE, **kw)
    LAST_EXEC_NS = res.exec_time_ns
    results = res.results
    LAST_RESULTS = results

    out = np.zeros([B, S, D], np.float32)
    attn = np.empty([B, H, S, S], np.float32)
    for c in range(NCORES):
        b, g = divmod(c, 4)
        out[b] += np.asarray(results[c]["out_p"], np.float32)
        at = np.asarray(results[c]["attn_t"])  # [4, k, q] bf16
        attn[b, g * 4:(g + 1) * 4] = at.transpose(0, 2, 1).astype(np.float32)
    wo_b = np.asarray(wo_b, np.float32)
    if np.any(wo_b != 0.0):
        out += wo_b
    return out, attn


# revision 48
# speedup vs baseline: 1.0444x; 1.0444x over previous
"""Trainium2 Bass kernel for a dense transformer self-attention block.

Problem: out = (softmax(QK^T/sqrt(dk) + mask) V) Wo + bo  with fused QKV proj.
  x [2, 2048, 1024], 16 heads, dk=64. Returns (out, attn).

Sharding (8 cores): core c handles batch b=c//4 and head-group g=c%4
(heads 4g..4g+3).  Pure data/tensor parallelism: no collectives; the
host sums the 4 output-projection partials per batch and reassembles
the attention probabilities (pure layout: transpose + dtype cast).

Device dataflow per core (all matmul operands bf16, accumulation f32):
  phase 1: qkT = [Wq'|Wk]^T x^T   (Wq' pre-scaled by 1/sqrt(dk) on host)
           V   = x Wv             (natural [k, dv] layout, + ones column)
  main, per (q-chunk, head-pair):
    scoresT[k,q] = K Q^T          (two heads packed in the PE array via
                                   row groups: dk=64 contraction each)
    E = exp(scoresT)              (ScalarE, straight from PSUM; max-
                                   subtraction skipped: |scores| <~ 30)
    blendT[dv+1, q] = [V|1]^T E   (ones column makes row 64 the softmax
                                   denominator - no partition reduce)
    r = 1/denom broadcast to 128 partitions (GpSimd)
    attn = E * r  -> HBM as [h, k, q] bf16 (host transposes to [q, k])
    blend = blendT * r
  out_partial[s, :] = blend^T Wo  -> HBM f32
"""

import os
import numpy as np
import ml_dtypes
from contextlib import ExitStack

B, S, D, H, DK = 2, 2048, 1024, 16, 64
NCORES = 8
HPC = 4  # heads per core
NEG_BIG = -float(2**63)
BF16 = ml_dtypes.bfloat16

PROFILE = False  # set True (e.g. from test.py) to neuron-profile the run
TRACE_DIR = None
LAST_EXEC_NS = None
LAST_RESULTS = None

_GRAPH_CACHE = {}


def _build_graph(use_mask, use_qk_bias, use_v_bias, debug_taps=False):
    import concourse.bass as bass
    import concourse.mybir as mybir
    import concourse.tile as tile
    from concourse import bacc

    DT = mybir.dt
    BF = DT.bfloat16
    F32 = DT.float32

    nc = bacc.Bacc("TRN2", target_bir_lowering=False, debug=False,
                   num_devices=NCORES)

    xt_d = nc.dram_tensor("xt", [D, S], BF, kind="ExternalInput")
    wqk_d = nc.dram_tensor("wqk", [D, 512], BF, kind="ExternalInput")
    wv_d = nc.dram_tensor("wv", [D, 256], BF, kind="ExternalInput")
    wo_d = nc.dram_tensor("wo", [256, D], BF, kind="ExternalInput")
    if use_mask:
        amb_d = nc.dram_tensor("amb", [128, 16], F32, kind="ExternalInput")
    if use_qk_bias:
        qkb_d = nc.dram_tensor("qkb", [128, 4], F32, kind="ExternalInput")
    if use_v_bias:
        vb_d = nc.dram_tensor("vb", [64, 4], F32, kind="ExternalInput")
    attn_d = nc.dram_tensor("attn_t", [HPC, S, S], BF, kind="ExternalOutput")
    outp_d = nc.dram_tensor("out_p", [S, D], F32, kind="ExternalOutput")
    if debug_taps:
        dbg_qkt = nc.dram_tensor("dbg_qkt", [4, 128, S], BF, kind="ExternalOutput")
        dbg_v = nc.dram_tensor("dbg_v", [128, 16, 264], BF, kind="ExternalOutput")
        dbg_exp = nc.dram_tensor("dbg_exp", [128, 16, 1024], BF,
                                 kind="ExternalOutput")
        dbg_den = nc.dram_tensor("dbg_den", [4, 512], F32, kind="ExternalOutput")
        dbg_rbc = nc.dram_tensor("dbg_rbc", [4, 128, 512], BF,
                                 kind="ExternalOutput")
        dbg_bl = nc.dram_tensor("dbg_bl", [4, 64, 512], BF, kind="ExternalOutput")

    EXP = mybir.ActivationFunctionType.Exp
    MUL = mybir.AluOpType.mult
    ADD = mybir.AluOpType.add

    with tile.TileContext(nc) as tc, ExitStack() as ctx:
        consts = ctx.enter_context(tc.tile_pool(name="consts", bufs=1))
        work = ctx.enter_context(tc.tile_pool(name="work", bufs=3))
        small = ctx.enter_context(tc.tile_pool(name="small", bufs=3))
        outs = ctx.enter_context(tc.tile_pool(name="outs", bufs=3))
        blpool = ctx.enter_context(tc.tile_pool(name="bl", bufs=2))
        xtp = ctx.enter_context(tc.tile_pool(name="xtp", bufs=24))
        # PSUM budget (8 banks): scores 2x2 + blend(psb) 3x1 + outproj 1 = 8
        # (phase-1 and broadcast matmuls borrow the scores slots)
        ps_s = ctx.enter_context(tc.tile_pool(name="ps_s", bufs=2, space="PSUM"))
        ps_b = ctx.enter_context(tc.tile_pool(name="ps_b", bufs=3, space="PSUM"))
        ps_x = ctx.enter_context(tc.tile_pool(name="ps_x", bufs=1, space="PSUM"))

        # ---- constant loads (issue order matters: first matmul group needs
        # wqk + wv + the sc=0 x^T chunks only) ---------------------------------
        wqk_sb = consts.tile([128, 8, 512], BF, tag="wqk")
        nc.sync.dma_start(wqk_sb[:], wqk_d.ap().rearrange("(o p) m -> p o m", p=128))
        wv_sb = consts.tile([128, 8, 256], BF, tag="wv")
        nc.sync.dma_start(wv_sb[:], wv_d.ap().rearrange("(o p) m -> p o m", p=128))
        # x^T in [dc][sc] chunks of [128, 512] so phase-1 matmuls can start
        # as soon as the first s-chunk lands
        xt_sb = {}
        for sc in range(4):
            for dc in range(8):
                t = xtp.tile([128, 512], BF, tag="xt", name=f"xt{dc}_{sc}")
                nc.sync.dma_start(t[:], xt_d.ap()[dc * 128:(dc + 1) * 128,
                                                  sc * 512:(sc + 1) * 512])
                xt_sb[(dc, sc)] = t
        wo_sb = consts.tile([128, 2, 1024], BF, tag="wo")
        nc.sync.dma_start(wo_sb[:], wo_d.ap().rearrange("(o p) m -> p o m", p=128))
        if use_mask:
            amb_sb = consts.tile([128, 16], F32, tag="amb")
            nc.sync.dma_start(amb_sb[:], amb_d.ap())
        if use_qk_bias:
            qkb_sb = consts.tile([128, 4], F32, tag="qkb")
            nc.sync.dma_start(qkb_sb[:], qkb_d.ap())
        if use_v_bias:
            vb_sb = consts.tile([64, 4], F32, tag="vb")
            nc.sync.dma_start(vb_sb[:], vb_d.ap())

        qT_sb = consts.tile([128, 2, S], BF, tag="qT")
        kT_sb = consts.tile([128, 2, S], BF, tag="kT")
        # v_aug layout: per head a 66-col strip: [V(64) | ones | pad]
        v_sb = consts.tile([128, 16, 264], BF, tag="v")
        nc.vector.memset(v_sb[:], 1.0)
        ones_sb = consts.tile([128, 128], BF, tag="ones")
        nc.vector.memset(ones_sb[:], 1.0)

        # ---- emission helpers ----------------------------------------------
        def qkT_group(mt, sc):
            """One [128, 512] chunk of qkT = [Wq'|Wk]^T x^T."""
            target = qT_sb if mt < 2 else kT_sb
            prr = mt % 2
            ps = ps_s.tile([128, 512], F32, tag="scores", name=f"p1_{mt}_{sc}")
            for dc in range(8):
                nc.tensor.matmul(
                    ps[:],
                    lhsT=wqk_sb[:, dc, mt * 128:(mt + 1) * 128],
                    rhs=xt_sb[(dc, sc)][:, :],
                    start=(dc == 0), stop=(dc == 7))
            dst = target[:, prr, sc * 512:(sc + 1) * 512]
            if use_qk_bias:
                nc.vector.tensor_scalar(
                    out=dst, in0=ps[:], scalar1=qkb_sb[:, mt:mt + 1],
                    scalar2=None, op0=ADD)
            else:
                nc.vector.tensor_copy(out=dst, in_=ps[:])

        def v_group(st):
            """One [128 k, 256] s-tile of V = x Wv into the v_aug layout."""
            ps = ps_s.tile([128, 256], F32, tag="scores", name=f"p1v_{st}")
            for dc in range(8):
                nc.tensor.matmul(
                    ps[:],
                    lhsT=xt_sb[(dc, st // 4)][:, (st % 4) * 128:(st % 4 + 1) * 128],
                    rhs=wv_sb[:, dc, :],
                    start=(dc == 0), stop=(dc == 7))
            dst = v_sb[:, st, :].rearrange("p (h c) -> p h c", h=4)[:, :, 0:64]
            srcv = ps[:].rearrange("p (h c) -> p h c", h=4)
            nc.vector.tensor_copy(out=dst, in_=srcv)

        def pair_scores_exp(qc, pr, attn_un, kc):
            """scoresT pair matmul (row-group packed) + exp for one k-chunk."""
            ps = ps_s.tile([128, 1024], F32, tag="scores",
                           name=f"sc_{qc}_{pr}_{kc}")
            for m in range(2):
                nc.tensor.matmul(
                    ps[:, m * 512:(m + 1) * 512],
                    lhsT=kT_sb[m * 64:(m + 1) * 64, pr,
                               kc * 128:(kc + 1) * 128],
                    rhs=qT_sb[m * 64:(m + 1) * 64, pr,
                              qc * 512:(qc + 1) * 512],
                    start=True, stop=True)
            bias = amb_sb[:, kc:kc + 1] if use_mask else 0.0
            nc.scalar.activation(out=attn_un[:, kc, :], in_=ps[:],
                                 func=EXP, bias=bias)

        def pair_av(pr, attn_un, psb, kc):
            """attn@V accumulation (V augmented with a ones column) for one
            k-chunk of a pair."""
            for m in range(2):
                h = 2 * pr + m
                nc.tensor.matmul(
                    psb[m][0:65, :],
                    lhsT=v_sb[:, kc, h * 66:h * 66 + 65],
                    rhs=attn_un[:, kc, m * 512:(m + 1) * 512],
                    start=(kc == 0), stop=(kc == 15))

        def pair_epilogue(qc, pr, attn_un, psb):
            """Denominator -> reciprocal broadcast; normalize + emit attn and
            blend.  Returns the pair's blend tile [128 hdv, 512 q]."""
            bp = blpool.tile([128, 512], BF, tag="bp", name=f"bp_{qc}_{pr}")
            # tiny lane-64 casts first so the PE broadcast matmuls are never
            # stuck behind a multi-us TENSOR_TENSOR on VectorE
            rbs, rbcs = [], []
            for m in range(2):
                rb = small.tile([128, 512], BF, tag="rb")
                nc.vector.tensor_copy(out=rb[64:65, :], in_=psb[m][64:65, :])
                rbs.append(rb)
            rbp = ps_s.tile([128, 1024], F32, tag="scores", name="rbp")
            for m in range(2):
                nc.tensor.matmul(rbp[:, m * 512:(m + 1) * 512],
                                 lhsT=ones_sb[64:65, 0:128],
                                 rhs=rbs[m][64:65, :], start=True, stop=True)
            for m in range(2):
                rc = small.tile([128, 512], F32, tag="rc")
                nc.vector.reciprocal_approx_fast(
                    out=rc[:, :], in_=rbp[:, m * 512:(m + 1) * 512])
                rbc = small.tile([128, 512], BF, tag="rbc")
                nc.vector.tensor_copy(out=rbc[:, :], in_=rc[:, :])
                rbcs.append(rbc)
                if debug_taps and qc == 0:
                    nc.sync.dma_start(out=dbg_den.ap()[2 * pr + m:
                                                       2 * pr + m + 1, :],
                                      in_=rc[0:1, :])
                    nc.sync.dma_start(out=dbg_rbc.ap()[2 * pr + m, :, :],
                                      in_=rbc[:, :])
            for m in range(2):
                h = 2 * pr + m
                rbc = rbcs[m]
                # normalize attn tiles in place, then write out (gpsimd queue
                # so bulk attn traffic never blocks small control DMAs)
                for half in range(2):
                    sl = attn_un[:, half * 8:(half + 1) * 8,
                                 m * 512:(m + 1) * 512]
                    nc.vector.tensor_tensor(
                        out=sl, in0=sl,
                        in1=rbc[:, None, :].to_broadcast((128, 8, 512)),
                        op=MUL)
                    nc.sync.dma_start(
                        out=attn_d.ap()[h].rearrange("(t p) q -> p t q", p=128)
                        [:, half * 8:(half + 1) * 8,
                         qc * 512:(qc + 1) * 512],
                        in_=sl)
                # normalize blended values (f32 from PSUM, -> bf16)
                if m == 0:
                    nc.vector.tensor_tensor(out=bp[0:64, :], in0=psb[m][0:64, :],
                                            in1=rbc[0:64, :], op=MUL)
                    if use_v_bias:
                        nc.vector.tensor_scalar(
                            out=bp[0:64, :], in0=bp[0:64, :],
                            scalar1=vb_sb[:, h:h + 1], scalar2=None, op0=ADD)
                else:
                    bt = small.tile([64, 512], BF, tag="btmp")
                    nc.vector.tensor_tensor(out=bt[:], in0=psb[m][0:64, :],
                                            in1=rbc[0:64, :], op=MUL)
                    if use_v_bias:
                        nc.vector.tensor_scalar(
                            out=bt[:], in0=bt[:], scalar1=vb_sb[:, h:h + 1],
                            scalar2=None, op0=ADD)
                    # lane shift to partitions 64-127 via DMA
                    nc.sync.dma_start(out=bp[64:128, :], in_=bt[:])
                if debug_taps and qc == 0:
                    nc.sync.dma_start(out=dbg_bl.ap()[h, :, :],
                                      in_=bp[m * 64:(m + 1) * 64, :])
            return bp

        def outproj_st(qc, bps, st):
            s0 = qc * 512 + st * 128
            out_sb = outs.tile([128, 1024], F32, tag="out")
            for nk in range(2):
                pso = ps_x.tile([128, 512], F32, tag="aux", name="pso")
                for prr in range(2):
                    nc.tensor.matmul(
                        pso[:],
                        lhsT=bps[prr][:, st * 128:(st + 1) * 128],
                        rhs=wo_sb[:, prr, nk * 512:(nk + 1) * 512],
                        start=(prr == 0), stop=(prr == 1))
                nc.vector.tensor_copy(out=out_sb[:, nk * 512:(nk + 1) * 512],
                                      in_=pso[:])
            nc.sync.dma_start(out=outp_d.ap()[s0:s0 + 128, :], in_=out_sb[:])

        def outproj(qc, bps):
            for st in range(4):
                s0 = qc * 512 + st * 128
                out_sb = outs.tile([128, 1024], F32, tag="out")
                for nk in range(2):
                    pso = ps_x.tile([128, 512], F32, tag="aux", name="pso")
                    for prr in range(2):
                        nc.tensor.matmul(
                            pso[:],
                            lhsT=bps[prr][:, st * 128:(st + 1) * 128],
                            rhs=wo_sb[:, prr, nk * 512:(nk + 1) * 512],
                            start=(prr == 0), stop=(prr == 1))
                    nc.vector.tensor_copy(out=out_sb[:, nk * 512:(nk + 1) * 512],
                                          in_=pso[:])
                nc.sync.dma_start(out=outp_d.ap()[s0:s0 + 128, :], in_=out_sb[:])

        # ---- software pipeline over the 8 (qc, pr) pairs --------------------
        # scores+exp of pair i run per-k-chunk-interleaved with attn@V of pair
        # i-1, so ScalarE's exp stream never waits for TensorE streaming or
        # the pair epilogues.  Pair 0 is additionally interleaved into phase 1
        # (its attn@V fused since the blend accumulators are free), and pair
        # 1's scores+exp fill ScalarE during the trailing phase-1 chunks.
        pairs = [(qc, pr) for qc in range(4) for pr in range(2)]
        au = {}
        psbs = {}
        bps = {}

        au[0] = work.tile([128, 16, 1024], BF, tag="attn_un", name="au0")
        psb_a0 = ps_b.tile([128, 512], F32, tag="blend_ps")
        psb_b0 = ps_b.tile([128, 512], F32, tag="blend_ps")
        psbs[0] = [psb_a0, psb_b0]
        au[1] = work.tile([128, 16, 1024], BF, tag="attn_un", name="au1")
        for sc in range(4):
            qkT_group(2, sc)
            qkT_group(3, sc)
            for st in range(sc * 4, sc * 4 + 4):
                v_group(st)
            if sc == 0:
                qkT_group(0, 0)
                qkT_group(1, 0)
            for kc in range(sc * 4, sc * 4 + 4):
                pair_scores_exp(0, 0, au[0], kc)
                if sc > 0:
                    # pair0's attn@V lags one s-chunk
                    pair_av(0, au[0], psbs[0], kc - 4)
            for kc in range(sc * 4, sc * 4 + 4):
                pair_scores_exp(0, 1, au[1], kc)
        for kc in range(12, 16):
            pair_av(0, au[0], psbs[0], kc)
        qkT_group(0, 1)
        qkT_group(1, 1)
        if debug_taps:
            for mt in range(4):
                tgt = qT_sb if mt < 2 else kT_sb
                nc.sync.dma_start(out=dbg_qkt.ap()[mt, :, :],
                                  in_=tgt[:, mt % 2, :])
            nc.sync.dma_start(out=dbg_v.ap()[:, :, :], in_=v_sb[:])

        bps[0] = pair_epilogue(0, 0, au[0], psbs[0])

        # block 2: scores/exp of pair 2 with the full attn@V of pair 1
        psb_a1 = ps_b.tile([128, 512], F32, tag="blend_ps")
        psb_b1 = ps_b.tile([128, 512], F32, tag="blend_ps")
        psbs[1] = [psb_a1, psb_b1]
        au[2] = work.tile([128, 16, 1024], BF, tag="attn_un", name="au2")
        for kc in range(16):
            pair_scores_exp(1, 0, au[2], kc)
            pair_av(pairs[1][1], au[1], psbs[1], kc)
        bps[1] = pair_epilogue(0, 1, au[1], psbs[1])
        outproj(0, [bps[0], bps[1]])

        # blocks 3..8 with half-block lag: in block i, pair i-1 finishes its
        # attn@V during the first half (epilogue + attn DMA fire mid-block),
        # pair i starts its own attn@V in the second half.
        def alloc_psb(idx):
            psb_a = ps_b.tile([128, 512], F32, tag="blend_ps",
                              name=f"psba{idx}")
            psb_b = ps_b.tile([128, 512], F32, tag="blend_ps",
                              name=f"psbb{idx}")
            psbs[idx] = [psb_a, psb_b]

        for i in range(3, 9):
            live = i < 8
            if live:
                qc, pr = pairs[i]
                au[i] = work.tile([128, 16, 1024], BF, tag="attn_un",
                                  name=f"au{i}")
            for j in range(16):
                if live:
                    pair_scores_exp(qc, pr, au[i], j)
                if j < 8:
                    # finish pair i-1's attn@V
                    if i == 3:
                        if j == 0:
                            alloc_psb(2)
                        pair_av(pairs[2][1], au[2], psbs[2], 2 * j)
                        pair_av(pairs[2][1], au[2], psbs[2], 2 * j + 1)
                    else:
                        pair_av(pairs[i - 1][1], au[i - 1], psbs[i - 1], j + 8)
                else:
                    if j == 8:
                        pqc, ppr = pairs[i - 1]
                        bps[i - 1] = pair_epilogue(pqc, ppr, au[i - 1],
                                                   psbs[i - 1])
                        if ppr == 1 and not live:
                            outproj(pqc, [bps[i - 2], bps[i - 1]])
                    if (live and pairs[i - 1][1] == 1 and j >= 8
                            and j % 2 == 0):
                        # spread the out-proj matmuls so they never delay the
                        # next scores chunk by a full burst
                        outproj_st(pairs[i - 1][0],
                                   [bps[i - 2], bps[i - 1]], (j - 8) // 2)
                    if live:
                        if j == 8:
                            alloc_psb(i)
                        pair_av(pr, au[i], psbs[i], j - 8)
                if live and j == 4 and i in (3, 5):
                    # lazily produce qT for the next q-chunk (PE slack)
                    qkT_group(0, (i + 1) // 2)
                    qkT_group(1, (i + 1) // 2)

    nc.compile()
    return nc


def _prep_inputs(x, attention_mask, wk_w, wk_b, wo_w):
    """Shard + lay out the full inputs for the 8 cores (host-side numpy)."""
    x = np.asarray(x, np.float32)
    wk_w = np.asarray(wk_w, np.float32)
    wk_b = np.asarray(wk_b, np.float32)
    wo_w = np.asarray(wo_w, np.float32)
    am = np.asarray(attention_mask, np.float32)

    amb = 1.0 - am.reshape(B, S)
    amb = np.where(amb == 1.0, np.float32(NEG_BIG), amb).astype(np.float32)
    use_mask = bool(np.any(amb != 0.0))
    use_qk_bias = bool(np.any(wk_b[:2 * D] != 0.0))
    use_v_bias = bool(np.any(wk_b[2 * D:] != 0.0))

    scale = np.float32(1.0 / np.sqrt(DK))
    xt = [np.ascontiguousarray(x[b].T).astype(BF16) for b in range(B)]

    in_maps = []
    for c in range(NCORES):
        b, g = divmod(c, 4)
        cols = slice(g * 256, (g + 1) * 256)
        wq = wk_w[:, cols] * scale
        wk = wk_w[:, 1024:2048][:, cols]
        wv = wk_w[:, 2048:3072][:, cols]
        m = {
            "xt": xt[b],
            "wqk": np.ascontiguousarray(
                np.concatenate([wq, wk], axis=1)).astype(BF16),
            "wv": np.ascontiguousarray(wv).astype(BF16),
            "wo": np.ascontiguousarray(wo_w[g * 256:(g + 1) * 256, :]).astype(BF16),
        }
        if use_mask:
            m["amb"] = np.ascontiguousarray(
                amb[b].reshape(16, 128).T).astype(np.float32)
        if use_qk_bias:
            qb = wk_b[cols] * scale
            kb = wk_b[1024:2048][cols]
            m["qkb"] = np.ascontiguousarray(np.stack(
                [qb[:128], qb[128:], kb[:128], kb[128:]], axis=1)).astype(np.float32)
        if use_v_bias:
            vb = wk_b[2048:3072][cols]
            m["vb"] = np.ascontiguousarray(
                vb.reshape(4, 64).T).astype(np.float32)
        in_maps.append(m)
    return in_maps, (use_mask, use_qk_bias, use_v_bias)


def kernel(x, attention_mask, wk_w, wk_b, wo_w, wo_b):
    global LAST_EXEC_NS, LAST_RESULTS
    from concourse.bass_utils import run_bass_kernel_spmd

    in_maps, variant = _prep_inputs(x, attention_mask, wk_w, wk_b, wo_w)
    if variant not in _GRAPH_CACHE:
        _GRAPH_CACHE[variant] = _build_graph(*variant)
    nc = _GRAPH_CACHE[variant]

    kw = {}
    if PROFILE and TRACE_DIR:
        kw["tmpdir"] = TRACE_DIR
    res = run_bass_kernel_spmd(nc, in_maps, core_ids=list(range(NCORES)),
                               trace=PROFILE, **kw)
    LAST_EXEC_NS = res.exec_time_ns
    results = res.results
    LAST_RESULTS = results

    out = np.zeros([B, S, D], np.float32)
    attn = np.empty([B, H, S, S], np.float32)
    for c in range(NCORES):
        b, g = divmod(c, 4)
        out[b] += np.asarray(results[c]["out_p"], np.float32)
        at = np.asarray(results[c]["attn_t"])  # [4, k, q] bf16
        attn[b, g * 4:(g + 1) * 4] = at.transpose(0, 2, 1).astype(np.float32)
    wo_b = np.asarray(wo_b, np.float32)
    if np.any(wo_b != 0.0):
        out += wo_b
    return out, attn


# revision 49
# speedup vs baseline: 1.0746x; 1.0290x over previous
"""Trainium2 Bass kernel for a dense transformer self-attention block.

Problem: out = (softmax(QK^T/sqrt(dk) + mask) V) Wo + bo  with fused QKV proj.
  x [2, 2048, 1024], 16 heads, dk=64. Returns (out, attn).

Sharding (8 cores): core c handles batch b=c//4 and head-group g=c%4
(heads 4g..4g+3).  Pure data/tensor parallelism: no collectives; the
host sums the 4 output-projection partials per batch and reassembles
the attention probabilities (pure layout: transpose + dtype cast).

Device dataflow per core (all matmul operands bf16, accumulation f32):
  phase 1: qkT = [Wq'|Wk]^T x^T   (Wq' pre-scaled by 1/sqrt(dk) on host)
           V   = x Wv             (natural [k, dv] layout, + ones column)
  main, per (q-chunk, head-pair):
    scoresT[k,q] = K Q^T          (two heads packed in the PE array via
                                   row groups: dk=64 contraction each)
    E = exp(scoresT)              (ScalarE, straight from PSUM; max-
                                   subtraction skipped: |scores| <~ 30)
    blendT[dv+1, q] = [V|1]^T E   (ones column makes row 64 the softmax
                                   denominator - no partition reduce)
    r = 1/denom broadcast to 128 partitions (GpSimd)
    attn = E * r  -> HBM as [h, k, q] bf16 (host transposes to [q, k])
    blend = blendT * r
  out_partial[s, :] = blend^T Wo  -> HBM f32
"""

import os
import numpy as np
import ml_dtypes
from contextlib import ExitStack

B, S, D, H, DK = 2, 2048, 1024, 16, 64
NCORES = 8
HPC = 4  # heads per core
NEG_BIG = -float(2**63)
BF16 = ml_dtypes.bfloat16

PROFILE = False  # set True (e.g. from test.py) to neuron-profile the run
TRACE_DIR = None
LAST_EXEC_NS = None
LAST_RESULTS = None

_GRAPH_CACHE = {}


def _build_graph(use_mask, use_qk_bias, use_v_bias, debug_taps=False):
    import concourse.bass as bass
    import concourse.mybir as mybir
    import concourse.tile as tile
    from concourse import bacc

    DT = mybir.dt
    BF = DT.bfloat16
    F32 = DT.float32

    nc = bacc.Bacc("TRN2", target_bir_lowering=False, debug=False,
                   num_devices=NCORES)

    xt_d = nc.dram_tensor("xt", [D, S], BF, kind="ExternalInput")
    wqk_d = nc.dram_tensor("wqk", [D, 512], BF, kind="ExternalInput")
    wv_d = nc.dram_tensor("wv", [D, 256], BF, kind="ExternalInput")
    wo_d = nc.dram_tensor("wo", [256, D], BF, kind="ExternalInput")
    if use_mask:
        amb_d = nc.dram_tensor("amb", [128, 16], F32, kind="ExternalInput")
    if use_qk_bias:
        qkb_d = nc.dram_tensor("qkb", [128, 4], F32, kind="ExternalInput")
    if use_v_bias:
        vb_d = nc.dram_tensor("vb", [64, 4], F32, kind="ExternalInput")
    attn_d = nc.dram_tensor("attn_t", [HPC, S, S], BF, kind="ExternalOutput")
    outp_d = nc.dram_tensor("out_p", [S, D], F32, kind="ExternalOutput")
    if debug_taps:
        dbg_qkt = nc.dram_tensor("dbg_qkt", [4, 128, S], BF, kind="ExternalOutput")
        dbg_v = nc.dram_tensor("dbg_v", [128, 16, 264], BF, kind="ExternalOutput")
        dbg_exp = nc.dram_tensor("dbg_exp", [128, 16, 1024], BF,
                                 kind="ExternalOutput")
        dbg_den = nc.dram_tensor("dbg_den", [4, 512], F32, kind="ExternalOutput")
        dbg_rbc = nc.dram_tensor("dbg_rbc", [4, 128, 512], BF,
                                 kind="ExternalOutput")
        dbg_bl = nc.dram_tensor("dbg_bl", [4, 64, 512], BF, kind="ExternalOutput")

    EXP = mybir.ActivationFunctionType.Exp
    MUL = mybir.AluOpType.mult
    ADD = mybir.AluOpType.add

    with tile.TileContext(nc) as tc, ExitStack() as ctx:
        consts = ctx.enter_context(tc.tile_pool(name="consts", bufs=1))
        work = ctx.enter_context(tc.tile_pool(name="work", bufs=3))
        small = ctx.enter_context(tc.tile_pool(name="small", bufs=3))
        outs = ctx.enter_context(tc.tile_pool(name="outs", bufs=3))
        blpool = ctx.enter_context(tc.tile_pool(name="bl", bufs=2))
        xtp = ctx.enter_context(tc.tile_pool(name="xtp", bufs=24))
        # PSUM budget (8 banks): scores 2x2 + blend(psb) 3x1 + outproj 1 = 8
        # (phase-1 and broadcast matmuls borrow the scores slots)
        ps_s = ctx.enter_context(tc.tile_pool(name="ps_s", bufs=2, space="PSUM"))
        ps_b = ctx.enter_context(tc.tile_pool(name="ps_b", bufs=3, space="PSUM"))
        ps_x = ctx.enter_context(tc.tile_pool(name="ps_x", bufs=1, space="PSUM"))

        # ---- constant loads (issue order matters: first matmul group needs
        # wqk + wv + the sc=0 x^T chunks only) ---------------------------------
        wqk_sb = consts.tile([128, 8, 512], BF, tag="wqk")
        nc.sync.dma_start(wqk_sb[:], wqk_d.ap().rearrange("(o p) m -> p o m", p=128))
        wv_sb = consts.tile([128, 8, 256], BF, tag="wv")
        nc.sync.dma_start(wv_sb[:], wv_d.ap().rearrange("(o p) m -> p o m", p=128))
        # x^T in [dc][sc] chunks of [128, 512] so phase-1 matmuls can start
        # as soon as the first s-chunk lands
        xt_sb = {}
        for sc in range(4):
            for dc in range(8):
                t = xtp.tile([128, 512], BF, tag="xt", name=f"xt{dc}_{sc}")
                nc.sync.dma_start(t[:], xt_d.ap()[dc * 128:(dc + 1) * 128,
                                                  sc * 512:(sc + 1) * 512])
                xt_sb[(dc, sc)] = t
        wo_sb = consts.tile([128, 2, 1024], BF, tag="wo")
        nc.sync.dma_start(wo_sb[:], wo_d.ap().rearrange("(o p) m -> p o m", p=128))
        if use_mask:
            amb_sb = consts.tile([128, 16], F32, tag="amb")
            nc.sync.dma_start(amb_sb[:], amb_d.ap())
        if use_qk_bias:
            qkb_sb = consts.tile([128, 4], F32, tag="qkb")
            nc.sync.dma_start(qkb_sb[:], qkb_d.ap())
        if use_v_bias:
            vb_sb = consts.tile([64, 4], F32, tag="vb")
            nc.sync.dma_start(vb_sb[:], vb_d.ap())

        qT_sb = consts.tile([128, 2, S], BF, tag="qT")
        kT_sb = consts.tile([128, 2, S], BF, tag="kT")
        # v_aug layout: per head a 66-col strip: [V(64) | ones | pad]
        v_sb = consts.tile([128, 16, 264], BF, tag="v")
        nc.vector.memset(v_sb[:], 1.0)
        ones_sb = consts.tile([128, 128], BF, tag="ones")
        nc.vector.memset(ones_sb[:], 1.0)

        # ---- emission helpers ----------------------------------------------
        def qkT_group(mt, sc):
            """One [128, 512] chunk of qkT = [Wq'|Wk]^T x^T."""
            target = qT_sb if mt < 2 else kT_sb
            prr = mt % 2
            ps = ps_s.tile([128, 512], F32, tag="scores", name=f"p1_{mt}_{sc}")
            for dc in range(8):
                nc.tensor.matmul(
                    ps[:],
                    lhsT=wqk_sb[:, dc, mt * 128:(mt + 1) * 128],
                    rhs=xt_sb[(dc, sc)][:, :],
                    start=(dc == 0), stop=(dc == 7))
            dst = target[:, prr, sc * 512:(sc + 1) * 512]
            if use_qk_bias:
                nc.vector.tensor_scalar(
                    out=dst, in0=ps[:], scalar1=qkb_sb[:, mt:mt + 1],
                    scalar2=None, op0=ADD)
            else:
                nc.vector.tensor_copy(out=dst, in_=ps[:])

        def v_group(st):
            """One [128 k, 256] s-tile of V = x Wv into the v_aug layout."""
            ps = ps_s.tile([128, 256], F32, tag="scores", name=f"p1v_{st}")
            for dc in range(8):
                nc.tensor.matmul(
                    ps[:],
                    lhsT=xt_sb[(dc, st // 4)][:, (st % 4) * 128:(st % 4 + 1) * 128],
                    rhs=wv_sb[:, dc, :],
                    start=(dc == 0), stop=(dc == 7))
            dst = v_sb[:, st, :].rearrange("p (h c) -> p h c", h=4)[:, :, 0:64]
            srcv = ps[:].rearrange("p (h c) -> p h c", h=4)
            nc.vector.tensor_copy(out=dst, in_=srcv)

        def pair_scores_exp(qc, pr, attn_un, kc):
            """scoresT pair matmul (row-group packed) + exp for one k-chunk."""
            ps = ps_s.tile([128, 1024], F32, tag="scores",
                           name=f"sc_{qc}_{pr}_{kc}")
            for m in range(2):
                nc.tensor.matmul(
                    ps[:, m * 512:(m + 1) * 512],
                    lhsT=kT_sb[m * 64:(m + 1) * 64, pr,
                               kc * 128:(kc + 1) * 128],
                    rhs=qT_sb[m * 64:(m + 1) * 64, pr,
                              qc * 512:(qc + 1) * 512],
                    start=True, stop=True)
            bias = amb_sb[:, kc:kc + 1] if use_mask else 0.0
            nc.scalar.activation(out=attn_un[:, kc, :], in_=ps[:],
                                 func=EXP, bias=bias)

        def pair_av(pr, attn_un, psb, kc):
            """attn@V accumulation (V augmented with a ones column) for one
            k-chunk of a pair."""
            for m in range(2):
                h = 2 * pr + m
                nc.tensor.matmul(
                    psb[m][0:65, :],
                    lhsT=v_sb[:, kc, h * 66:h * 66 + 65],
                    rhs=attn_un[:, kc, m * 512:(m + 1) * 512],
                    start=(kc == 0), stop=(kc == 15))

        def pair_epilogue(qc, pr, attn_un, psb):
            """Denominator -> reciprocal broadcast; normalize + emit attn and
            blend.  Returns the pair's blend tile [128 hdv, 512 q]."""
            bp = blpool.tile([128, 512], BF, tag="bp", name=f"bp_{qc}_{pr}")
            # tiny lane-64 casts first so the PE broadcast matmuls are never
            # stuck behind a multi-us TENSOR_TENSOR on VectorE
            rbs, rbcs = [], []
            for m in range(2):
                rb = small.tile([128, 512], BF, tag="rb")
                nc.vector.tensor_copy(out=rb[64:65, :], in_=psb[m][64:65, :])
                rbs.append(rb)
            rbp = ps_s.tile([128, 1024], F32, tag="scores", name="rbp")
            for m in range(2):
                nc.tensor.matmul(rbp[:, m * 512:(m + 1) * 512],
                                 lhsT=ones_sb[64:65, 0:128],
                                 rhs=rbs[m][64:65, :], start=True, stop=True)
            for m in range(2):
                rc = small.tile([128, 512], F32, tag="rc")
                nc.vector.reciprocal_approx_fast(
                    out=rc[:, :], in_=rbp[:, m * 512:(m + 1) * 512])
                rbc = small.tile([128, 512], BF, tag="rbc")
                nc.vector.tensor_copy(out=rbc[:, :], in_=rc[:, :])
                rbcs.append(rbc)
                if debug_taps and qc == 0:
                    nc.sync.dma_start(out=dbg_den.ap()[2 * pr + m:
                                                       2 * pr + m + 1, :],
                                      in_=rc[0:1, :])
                    nc.sync.dma_start(out=dbg_rbc.ap()[2 * pr + m, :, :],
                                      in_=rbc[:, :])
            for m in range(2):
                h = 2 * pr + m
                rbc = rbcs[m]
                # normalize attn tiles in place, then write out (gpsimd queue
                # so bulk attn traffic never blocks small control DMAs)
                for qtr in range(4):
                    sl = attn_un[:, qtr * 4:(qtr + 1) * 4,
                                 m * 512:(m + 1) * 512]
                    nc.vector.tensor_tensor(
                        out=sl, in0=sl,
                        in1=rbc[:, None, :].to_broadcast((128, 4, 512)),
                        op=MUL)
                    nc.sync.dma_start(
                        out=attn_d.ap()[h].rearrange("(t p) q -> p t q", p=128)
                        [:, qtr * 4:(qtr + 1) * 4,
                         qc * 512:(qc + 1) * 512],
                        in_=sl)
                # normalize blended values (f32 from PSUM, -> bf16)
                if m == 0:
                    nc.vector.tensor_tensor(out=bp[0:64, :], in0=psb[m][0:64, :],
                                            in1=rbc[0:64, :], op=MUL)
                    if use_v_bias:
                        nc.vector.tensor_scalar(
                            out=bp[0:64, :], in0=bp[0:64, :],
                            scalar1=vb_sb[:, h:h + 1], scalar2=None, op0=ADD)
                else:
                    bt = small.tile([64, 512], BF, tag="btmp")
                    nc.vector.tensor_tensor(out=bt[:], in0=psb[m][0:64, :],
                                            in1=rbc[0:64, :], op=MUL)
                    if use_v_bias:
                        nc.vector.tensor_scalar(
                            out=bt[:], in0=bt[:], scalar1=vb_sb[:, h:h + 1],
                            scalar2=None, op0=ADD)
                    # lane shift to partitions 64-127 via DMA
                    nc.sync.dma_start(out=bp[64:128, :], in_=bt[:])
                if debug_taps and qc == 0:
                    nc.sync.dma_start(out=dbg_bl.ap()[h, :, :],
                                      in_=bp[m * 64:(m + 1) * 64, :])
            return bp

        def outproj_st(qc, bps, st):
            s0 = qc * 512 + st * 128
            out_sb = outs.tile([128, 1024], F32, tag="out")
            for nk in range(2):
                pso = ps_x.tile([128, 512], F32, tag="aux", name="pso")
                for prr in range(2):
                    nc.tensor.matmul(
                        pso[:],
                        lhsT=bps[prr][:, st * 128:(st + 1) * 128],
                        rhs=wo_sb[:, prr, nk * 512:(nk + 1) * 512],
                        start=(prr == 0), stop=(prr == 1))
                nc.vector.tensor_copy(out=out_sb[:, nk * 512:(nk + 1) * 512],
                                      in_=pso[:])
            nc.sync.dma_start(out=outp_d.ap()[s0:s0 + 128, :], in_=out_sb[:])

        def outproj(qc, bps):
            for st in range(4):
                s0 = qc * 512 + st * 128
                out_sb = outs.tile([128, 1024], F32, tag="out")
                for nk in range(2):
                    pso = ps_x.tile([128, 512], F32, tag="aux", name="pso")
                    for prr in range(2):
                        nc.tensor.matmul(
                            pso[:],
                            lhsT=bps[prr][:, st * 128:(st + 1) * 128],
                            rhs=wo_sb[:, prr, nk * 512:(nk + 1) * 512],
                            start=(prr == 0), stop=(prr == 1))
                    nc.vector.tensor_copy(out=out_sb[:, nk * 512:(nk + 1) * 512],
                                          in_=pso[:])
                nc.sync.dma_start(out=outp_d.ap()[s0:s0 + 128, :], in_=out_sb[:])

        # ---- software pipeline over the 8 (qc, pr) pairs --------------------
        # scores+exp of pair i run per-k-chunk-interleaved with attn@V of pair
        # i-1, so ScalarE's exp stream never waits for TensorE streaming or
        # the pair epilogues.  Pair 0 is additionally interleaved into phase 1
        # (its attn@V fused since the blend accumulators are free), and pair
        # 1's scores+exp fill ScalarE during the trailing phase-1 chunks.
        pairs = [(qc, pr) for qc in range(4) for pr in range(2)]
        au = {}
        psbs = {}
        bps = {}

        au[0] = work.tile([128, 16, 1024], BF, tag="attn_un", name="au0")
        psb_a0 = ps_b.tile([128, 512], F32, tag="blend_ps")
        psb_b0 = ps_b.tile([128, 512], F32, tag="blend_ps")
        psbs[0] = [psb_a0, psb_b0]
        au[1] = work.tile([128, 16, 1024], BF, tag="attn_un", name="au1")
        for sc in range(4):
            qkT_group(2, sc)
            qkT_group(3, sc)
            for st in range(sc * 4, sc * 4 + 4):
                v_group(st)
            if sc == 0:
                qkT_group(0, 0)
                qkT_group(1, 0)
            for kc in range(sc * 4, sc * 4 + 4):
                pair_scores_exp(0, 0, au[0], kc)
                if sc > 0:
                    # pair0's attn@V lags one s-chunk
                    pair_av(0, au[0], psbs[0], kc - 4)
            for kc in range(sc * 4, sc * 4 + 4):
                pair_scores_exp(0, 1, au[1], kc)
        for kc in range(12, 16):
            pair_av(0, au[0], psbs[0], kc)
        qkT_group(0, 1)
        qkT_group(1, 1)
        if debug_taps:
            for mt in range(4):
                tgt = qT_sb if mt < 2 else kT_sb
                nc.sync.dma_start(out=dbg_qkt.ap()[mt, :, :],
                                  in_=tgt[:, mt % 2, :])
            nc.sync.dma_start(out=dbg_v.ap()[:, :, :], in_=v_sb[:])

        bps[0] = pair_epilogue(0, 0, au[0], psbs[0])

        # block 2: scores/exp of pair 2 with the full attn@V of pair 1
        psb_a1 = ps_b.tile([128, 512], F32, tag="blend_ps")
        psb_b1 = ps_b.tile([128, 512], F32, tag="blend_ps")
        psbs[1] = [psb_a1, psb_b1]
        au[2] = work.tile([128, 16, 1024], BF, tag="attn_un", name="au2")
        for kc in range(16):
            pair_scores_exp(1, 0, au[2], kc)
            pair_av(pairs[1][1], au[1], psbs[1], kc)
        bps[1] = pair_epilogue(0, 1, au[1], psbs[1])
        outproj(0, [bps[0], bps[1]])

        # blocks 3..8 with half-block lag: in block i, pair i-1 finishes its
        # attn@V during the first half (epilogue + attn DMA fire mid-block),
        # pair i starts its own attn@V in the second half.
        def alloc_psb(idx):
            psb_a = ps_b.tile([128, 512], F32, tag="blend_ps",
                              name=f"psba{idx}")
            psb_b = ps_b.tile([128, 512], F32, tag="blend_ps",
                              name=f"psbb{idx}")
            psbs[idx] = [psb_a, psb_b]

        for i in range(3, 9):
            live = i < 8
            if live:
                qc, pr = pairs[i]
                au[i] = work.tile([128, 16, 1024], BF, tag="attn_un",
                                  name=f"au{i}")
            for j in range(16):
                if live:
                    pair_scores_exp(qc, pr, au[i], j)
                if j < 8:
                    # finish pair i-1's attn@V
                    if i == 3:
                        if j == 0:
                            alloc_psb(2)
                        pair_av(pairs[2][1], au[2], psbs[2], 2 * j)
                        pair_av(pairs[2][1], au[2], psbs[2], 2 * j + 1)
                    else:
                        pair_av(pairs[i - 1][1], au[i - 1], psbs[i - 1], j + 8)
                else:
                    if j == 8:
                        pqc, ppr = pairs[i - 1]
                        bps[i - 1] = pair_epilogue(pqc, ppr, au[i - 1],
                                                   psbs[i - 1])
                        if ppr == 1 and not live:
                            outproj(pqc, [bps[i - 2], bps[i - 1]])
                    if (live and pairs[i - 1][1] == 1 and j >= 8
                            and j % 2 == 0):
                        # spread the out-proj matmuls so they never delay the
                        # next scores chunk by a full burst
                        outproj_st(pairs[i - 1][0],
                                   [bps[i - 2], bps[i - 1]], (j - 8) // 2)
                    if live:
                        if j == 8:
                            alloc_psb(i)
                        pair_av(pr, au[i], psbs[i], j - 8)
                if live and j == 4 and i in (3, 5):
                    # lazily produce qT for the next q-chunk (PE slack)
                    qkT_group(0, (i + 1) // 2)
                    qkT_group(1, (i + 1) // 2)

    nc.compile()
    return nc


def _prep_inputs(x, attention_mask, wk_w, wk_b, wo_w):
    """Shard + lay out the full inputs for the 8 cores (host-side numpy)."""
    x = np.asarray(x, np.float32)
    wk_w = np.asarray(wk_w, np.float32)
    wk_b = np.asarray(wk_b, np.float32)
    wo_w = np.asarray(wo_w, np.float32)
    am = np.asarray(attention_mask, np.float32)

    amb = 1.0 - am.reshape(B, S)
    amb = np.where(amb == 1.0, np.float32(NEG_BIG), amb).astype(np.float32)
    use_mask = bool(np.any(amb != 0.0))
    use_qk_bias = bool(np.any(wk_b[:2 * D] != 0.0))
    use_v_bias = bool(np.any(wk_b[2 * D:] != 0.0))

    scale = np.float32(1.0 / np.sqrt(DK))
    xt = [np.ascontiguousarray(x[b].T).astype(BF16) for b in range(B)]

    in_maps = []
    for c in range(NCORES):
        b, g = divmod(c, 4)
        cols = slice(g * 256, (g + 1) * 256)
        wq = wk_w[:, cols] * scale
        wk = wk_w[:, 1024:2048][:, cols]
        wv = wk_w[:, 2048:3072][:, cols]
        m = {
            "xt": xt[b],
            "wqk": np.ascontiguousarray(
                np.concatenate([wq, wk], axis=1)).astype(BF16),
            "wv": np.ascontiguousarray(wv).astype(BF16),
            "wo": np.ascontiguousarray(wo_w[g * 256:(g + 1) * 256, :]).astype(BF16),
        }
        if use_mask:
            m["amb"] = np.ascontiguousarray(
                amb[b].reshape(16, 128).T).astype(np.float32)
        if use_qk_bias:
            qb = wk_b[cols] * scale
            kb = wk_b[1024:2048][cols]
            m["qkb"] = np.ascontiguousarray(np.stack(
                [qb[:128], qb[128:], kb[:128], kb[128:]], axis=1)).astype(np.float32)
        if use_v_bias:
            vb = wk_b[2048:3072][cols]
            m["vb"] = np.ascontiguousarray(
                vb.reshape(4, 64).T).astype(np.float32)
        in_maps.append(m)
    return in_maps, (use_mask, use_qk_bias, use_v_bias)


def kernel(x, attention_mask, wk_w, wk_b, wo_w, wo_b):
    global LAST_EXEC_NS, LAST_RESULTS
    from concourse.bass_utils import run_bass_kernel_spmd

    in_maps, variant = _prep_inputs(x, attention_mask, wk_w, wk_b, wo_w)
    if variant not in _GRAPH_CACHE:
        _GRAPH_CACHE[variant] = _build_graph(*variant)
    nc = _GRAPH_CACHE[variant]

    kw = {}
    if PROFILE and TRACE_DIR:
        kw["tmpdir"] = TRACE_DIR
    res = run_bass_kernel_spmd(nc, in_maps, core_ids=list(range(NCORES)),
                               trace=PROFILE, **kw)
    LAST_EXEC_NS = res.exec_time_ns
    results = res.results
    LAST_RESULTS = results

    out = np.zeros([B, S, D], np.float32)
    attn = np.empty([B, H, S, S], np.float32)
    for c in range(NCORES):
        b, g = divmod(c, 4)
        out[b] += np.asarray(results[c]["out_p"], np.float32)
        at = np.asarray(results[c]["attn_t"])  # [4, k, q] bf16
        attn[b, g * 4:(g + 1) * 4] = at.transpose(0, 2, 1).astype(np.float32)
    wo_b = np.asarray(wo_b, np.float32)
    if np.any(wo_b != 0.0):
        out += wo_b
    return out, attn
